# revision 17
# baseline (speedup 1.0000x reference)
"""Trainium2 Bass kernel for nn_CrossAttention (4-layer MLP -> cross-attention).

Sharding: data-parallel across batch B=8, one batch element per NeuronCore.

Layout strategy (per core):
  - activations flow feature-major (transposed): the MLP chain
    h_{l+1}^T = W_l^T @ h_l^T needs no transposes (W natural [K,M] = lhsT),
  - scores are computed transposed (scores^T = k @ q^T, kv on partitions) so
    Em1 = exp(scores^T)-1 feeds the attention output matmul directly as the
    stationary operand: out[q,D] = lhsT(Em1[kv,q]).T @ v8[kv,D],
  - softmax skips max-subtraction (exact by shift invariance; scores small).

Precision: ALL five GEMM groups (x-MLP/q/k, v, scores, attn@v) run in
fp8(e4m3) DoubleRow (2 contraction rows per PE pass -> 2x rate). The
attention weights E ~ 1 +- 0.1 cannot survive fp8 directly (6% steps at 1.0),
so the output is decomposed around the constant term:
    out = (sum_kv (E-1) * v8  +  sum_kv v) / (2048 + sum_kv (E-1))
The fluctuation term (E-1) ~ 0.1 and v8 are fp8 (their product errors average
over Skv=2048), while the dominant sum_kv v term is computed exactly in bf16
as (sum_kv y) @ Wv + 2048*bv via a DVE row-reduction of y^T and a small
rank-1 matmul chain. Row sums come from DR matmuls with a ones column
(sharing the attention matmuls' loaded weights); the +2048 and the 1/S
normalize happen on DVE, the final scale on the scalar engine.
All accumulation fp32 in PSUM. Output stored bf16, upcast on host.

fp8 operands are pair-packed for DoubleRow: logical feature k = (2t+r)*128+p
lives in tile t, partition p, middle index r, i.e. SBUF tiles [128, 2, N]
(weights pre-packed on host to [K/2, 2*N] so each tile is one contiguous DMA).

Drains are split between the scalar and vector engines on layers where the
PE would otherwise outrun a single drain engine (L2/L3/k: only 2-4 DR passes
per psum tile).

Everything is SBUF-resident (no DRAM round-trips); y/Wk prefetch from t=0.
"""

import sys

if "/opt/trn_rl_repo" not in sys.path:
    sys.path.insert(0, "/opt/trn_rl_repo")

import numpy as np
import ml_dtypes

P = 128
D = 1024
DB = 512
S = 2048
KD = D // P       # 8 feature tiles of 128
KB = DB // P      # 4
PD = KD // 2      # 4 fp8 pair-tiles for a 1024 contraction
PB = KB // 2      # 2 for 512
NT = S // P       # 16 token tiles
NP = NT // 2      # 8 kv pair-tiles for the attention contraction
NB = 512          # moving-operand free-dim block (psum bank width)
NBLK = S // NB    # 4 token blocks
NCORES = 8
SCALE = float(1.0 / np.sqrt(D))

BF16 = ml_dtypes.bfloat16
FP8 = ml_dtypes.float8_e4m3

_NC = None


def build_nc():
    """Build + compile the per-core Bass program (cached)."""
    global _NC
    if _NC is not None:
        return _NC

    from contextlib import ExitStack
    import concourse.bass as bass
    import concourse.tile as tile
    from concourse import bacc, mybir

    BF = mybir.dt.bfloat16
    F8 = mybir.dt.float8e4
    F32 = mybir.dt.float32
    AF = mybir.ActivationFunctionType
    ALU = mybir.AluOpType
    DR = mybir.MatmulPerfMode.DoubleRow

    nc = bacc.Bacc("TRN2", target_bir_lowering=False, debug=False,
                   num_devices=NCORES)

    def din(name, shape, dt):
        return nc.dram_tensor(name, shape, dt, kind="ExternalInput").ap()

    # fp8 operands arrive pair-packed: [K/2, 2*N]
    x8d = din("x8", [D // 2, 2 * S], F8)
    y8d = din("y8", [D // 2, 2 * S], F8)
    yTd = din("yT", [D, S], BF)
    W1d = din("W1", [D // 2, 2 * D], F8)
    W2d = din("W2", [D // 2, 2 * DB], F8)
    W3d = din("W3", [DB // 2, 2 * D], F8)
    W4d = din("W4", [D // 2, 2 * D], F8)
    Wqd = din("Wq", [D // 2, 2 * D], F8)
    Wkd = din("Wk", [D // 2, 2 * D], F8)
    Wv8d = din("Wv8", [D // 2, 2 * D], F8)
    Wvd = din("Wv", [D, D], BF)
    b1 = din("b1", [P, KD], F32)
    b2 = din("b2", [P, KB], F32)
    b3 = din("b3", [P, KD], F32)
    b4 = din("b4", [P, KD], F32)
    bq = din("bq", [P, KD], F32)
    bk = din("bk", [P, KD], F32)
    bv = din("bv", [D], F32)
    bvs = din("bvs", [D], F32)          # 2048 * bv (host-precomputed)
    out = nc.dram_tensor("out", [S, D], BF, kind="ExternalOutput").ap()

    with tile.TileContext(nc) as tc, ExitStack() as ctx:
        small = ctx.enter_context(tc.tile_pool(name="small", bufs=1))
        rpool = ctx.enter_context(tc.tile_pool(name="rpool", bufs=4))
        outp = ctx.enter_context(tc.tile_pool(name="outp", bufs=2))
        tmpp = ctx.enter_context(tc.tile_pool(name="tmpp", bufs=2))

        def load_bias(src, cols, tag):
            t = small.tile([P, cols], F32, tag=tag, name=tag)
            nc.gpsimd.dma_start(out=t, in_=src)
            return t

        b1_sb = load_bias(b1, KD, "b1")
        b2_sb = load_bias(b2, KB, "b2")
        b3_sb = load_bias(b3, KD, "b3")
        b4_sb = load_bias(b4, KD, "b4")
        bq_sb = load_bias(bq, KD, "bq")
        bk_sb = load_bias(bk, KD, "bk")

        # bv replicated across partitions for the v8 bias add; allocated here,
        # DMA'd at stage B start (keeps the gpsimd queue free for W1 at t=0)
        bv_rep = small.tile([P, D], F32, tag="bvrep", name="bvrep")
        bv_bcast = bass.AP(tensor=bv.tensor, offset=bv.offset,
                           ap=[[0, P]] + list(bv.ap))
        # 2048*bv as a single row (for the exact sum_kv v term)
        bvs_row = small.tile([1, D], F32, tag="bvsrow", name="bvsrow")
        bvs_bcast = bass.AP(tensor=bvs.tensor, offset=bvs.offset,
                            ap=[[0, 1]] + list(bvs.ap))

        # fp8 pair-packed ones column for DR row sums
        ones8 = small.tile([P, 2, 1], F8, tag="ones8", name="ones8")
        nc.vector.memset(ones8, 1.0)
        # bf16 ones row for the sumv partition broadcast
        ones_r = small.tile([1, P], BF, tag="onesr", name="onesr")
        nc.vector.memset(ones_r, 1.0)
        warm_row = small.tile([1, NB], BF, tag="warm", name="warm")
        nc.vector.memset(warm_row, 0.0)

        def alloc_pairs(pool, pairs, n, tag, dt=F8):
            """fp8 pair-packed tiles [P, 2, n]."""
            return [pool.tile([P, 2, n], dt, tag=f"{tag}{t}", name=f"{tag}{t}")
                    for t in range(pairs)]

        def load_pairs(tiles, src, n, eng=None):
            # weights ride the gpsimd DGE queue so they stream in parallel
            # with the activation loads on the sync queue
            eng = eng if eng is not None else nc.gpsimd
            for t, tl in enumerate(tiles):
                eng.dma_start(
                    out=tl,
                    in_=src[t * P:(t + 1) * P, :].rearrange(
                        "p (r s) -> p r s", r=2))

        def fm_layer8(psum, src8, w8, pairs, mtiles, bias_sb, relu, dst8,
                      cw, dve_mod=0):
            """fp8 DoubleRow feature-major layer into pair-packed fp8 dst.

            cw: psum chunk width in tokens (512 or 1024). Chunks with
            (chunk_idx % 2) < dve_mod drain on the vector engine instead of
            the scalar engine (used where PE passes per chunk are few)."""
            func = AF.Relu if relu else AF.Identity
            nch = S // cw
            nsub = cw // NB
            for m in range(mtiles):
                pss = [psum.tile([P, cw], F32, tag="mm", name="mm")
                       for _ in range(nch)]
                for t in range(pairs):
                    lhs = w8[t][:, :, m * P:(m + 1) * P]
                    for c in range(nch):
                        for nb in range(nsub):
                            col = c * cw + nb * NB
                            nc.tensor.matmul(
                                pss[c][:, nb * NB:(nb + 1) * NB], lhs,
                                src8[t][:, :, col:col + NB],
                                start=(t == 0), stop=(t == pairs - 1),
                                perf_mode=DR)
                for c in range(nch):
                    dst = dst8[m // 2][:, m % 2, c * cw:(c + 1) * cw]
                    if c % 2 < dve_mod:
                        if relu:
                            nc.vector.tensor_scalar(
                                dst, pss[c], bias_sb[:, m:m + 1], 0.0,
                                op0=ALU.add, op1=ALU.max)
                        else:
                            nc.vector.tensor_scalar_add(
                                dst, pss[c], bias_sb[:, m:m + 1])
                    else:
                        nc.scalar.activation(dst, pss[c], func,
                                             bias=bias_sb[:, m:m + 1],
                                             scale=1.0)

        # ------ persistent attention operands (q8, k8, v8) + y prefetch ------
        with tc.tile_pool(name="pq", bufs=1) as pq, \
             tc.tile_pool(name="pk", bufs=1) as pk, \
             tc.tile_pool(name="pvp", bufs=1) as pvp, \
             tc.tile_pool(name="py", bufs=1) as py, \
             tc.tile_pool(name="pwk", bufs=1) as pwk, \
             tc.tile_pool(name="psv", bufs=1) as psv:
            q8 = alloc_pairs(pq, PD, S, "q8")
            k8 = alloc_pairs(pk, PD, S, "k8")
            v8 = alloc_pairs(pvp, NP, D, "v8")
            y8 = alloc_pairs(py, PD, S, "y8")
            wk8 = alloc_pairs(pwk, PD, D, "wk8")
            sumv_rep = psv.tile([P, D], F32, tag="svr", name="svr")

            # ---------------- Stage A: x-MLP -> q8 (in SBUF) ----------------
            with tc.tile_pool(name="wx", bufs=1) as wx, \
                 tc.tile_pool(name="px", bufs=1) as px, \
                 tc.tile_pool(name="phA", bufs=1) as phA, \
                 tc.tile_pool(name="phB", bufs=1) as phB, \
                 tc.tile_pool(name="psA", bufs=4, space="PSUM") as psA:
                x8 = alloc_pairs(px, PD, S, "x8")
                w18 = alloc_pairs(wx, PD, D, "w18")
                # first-needed tiles first: interleave x8 / W1 pair loads;
                # t=0 tiles split in half so the first matmul's deps land fast
                for t in range(PD):
                    xs = x8d[t * P:(t + 1) * P, :].rearrange(
                        "p (r s) -> p r s", r=2)
                    ws = W1d[t * P:(t + 1) * P, :].rearrange(
                        "p (r s) -> p r s", r=2)
                    if t == 0:
                        nc.sync.dma_start(out=x8[t][:, :, 0:NB],
                                          in_=xs[:, :, 0:NB])
                        nc.gpsimd.dma_start(out=w18[t][:, :, 0:NB],
                                            in_=ws[:, :, 0:NB])
                        nc.sync.dma_start(out=x8[t][:, :, NB:S],
                                          in_=xs[:, :, NB:S])
                        nc.gpsimd.dma_start(out=w18[t][:, :, NB:D],
                                            in_=ws[:, :, NB:D])
                    else:
                        nc.sync.dma_start(out=x8[t], in_=xs)
                        nc.gpsimd.dma_start(out=w18[t], in_=ws)
                w28 = alloc_pairs(wx, PD, DB, "w28")
                load_pairs(w28, W2d, DB)
                w38 = alloc_pairs(wx, PB, D, "w38")
                load_pairs(w38, W3d, D)
                w48 = alloc_pairs(wx, PD, D, "w48")
                load_pairs(w48, W4d, D)
                wq8 = alloc_pairs(wx, PD, D, "wq8")
                load_pairs(wq8, Wqd, D)
                # y-side prefetch (queued behind stage A's needs)
                load_pairs(y8, y8d, S, eng=nc.sync)
                load_pairs(wk8, Wkd, D)

                # warmup matmuls on preloaded constants: start the PE p-state
                # ramp while the first x8/W1 DMAs are still in flight
                warm_ps = psA.tile([P, 2 * NB], F32, tag="mm", name="warm_ps")
                for _ in range(9):
                    nc.tensor.matmul(warm_ps[:, 0:NB], ones_r, warm_row,
                                     start=True, stop=True)

                h18 = alloc_pairs(phA, PD, S, "ha")
                h28 = alloc_pairs(phB, PB, S, "hb")
                h38 = alloc_pairs(phA, PD, S, "ha")   # reuse phA slots
                h48 = alloc_pairs(phB, PD, S, "hb")   # grow phB to 4 pair slots
                fm_layer8(psA, x8, w18, PD, KD, b1_sb, True, h18, 2 * NB)
                fm_layer8(psA, h18, w28, PD, KB, b2_sb, True, h28, 2 * NB,
                          dve_mod=1)
                fm_layer8(psA, h28, w38, PB, KD, b3_sb, True, h38, 2 * NB,
                          dve_mod=1)
                fm_layer8(psA, h38, w48, PD, KD, b4_sb, True, h48, 2 * NB)
                fm_layer8(psA, h48, wq8, PD, KD, bq_sb, False, q8, 2 * NB)

            # ------- Stage B: y -> k8, v8 (fp8), exact sumv row (bf16) -------
            with tc.tile_pool(name="pwv", bufs=1) as pwv, \
                 tc.tile_pool(name="pys", bufs=1) as pys, \
                 tc.tile_pool(name="psBk", bufs=4, space="PSUM") as psBk, \
                 tc.tile_pool(name="psBv", bufs=2, space="PSUM") as psBv:
                # stage-B-only loads (deferred so stage A has SBUF headroom)
                nc.gpsimd.dma_start(out=bv_rep, in_=bv_bcast)
                nc.gpsimd.dma_start(out=bvs_row, in_=bvs_bcast)
                wv8 = alloc_pairs(pwv, PD, D, "wv8")
                load_pairs(wv8, Wv8d, D)
                ys = [pys.tile([P, S], BF, tag=f"y{k}", name=f"y{k}")
                      for k in range(KD)]
                for k in range(KD):
                    nc.sync.dma_start(out=ys[k], in_=yTd[k * P:(k + 1) * P, :])
                wvs = [pys.tile([P, D], BF, tag=f"wv{k}", name=f"wv{k}")
                       for k in range(KD)]
                for k in range(KD):
                    nc.gpsimd.dma_start(out=wvs[k],
                                        in_=Wvd[k * P:(k + 1) * P, :])

                # k^T in fp8 pairs; narrow psum chunks, drains split act/DVE
                fm_layer8(psBk, y8, wk8, PD, KD, bk_sb, False, k8, NB,
                          dve_mod=1)

                # v8[kv, d] = fp8(y @ Wv + bv), token-major pair-packed via DR
                for tk in range(NT):
                    pv_ = psBv.tile([P, D], F32, tag="vv", name="vv")
                    for t in range(PD):
                        lhs = y8[t][:, :, tk * P:(tk + 1) * P]
                        for nb in range(2):
                            nc.tensor.matmul(pv_[:, nb * NB:(nb + 1) * NB],
                                             lhs,
                                             wv8[t][:, :, nb * NB:(nb + 1) * NB],
                                             start=(t == 0), stop=(t == PD - 1),
                                             perf_mode=DR)
                    nc.vector.tensor_add(v8[tk // 2][:, tk % 2, :], pv_, bv_rep)

                # exact sum_kv v: ysumT = rowsum(y^T) via scalar-engine Copy
                # with accum_out (the DVE is busy with v8 drains; the scalar
                # engine idles during the v8 phase), then
                # sumv = ysumT @ Wv + 2048*bv (bf16 rank-1 chain)
                ydump = pys.tile([P, S], BF, tag="ydump", name="ydump")
                ysf = [rpool.tile([P, 1], F32, tag=f"ysf{k}", name=f"ysf{k}",
                                  bufs=1)
                       for k in range(KD)]
                ysb = [rpool.tile([P, 1], BF, tag=f"ysb{k}", name=f"ysb{k}",
                                  bufs=1)
                       for k in range(KD)]
                for k in range(KD):
                    nc.scalar.activation(ydump, ys[k], AF.Copy,
                                         accum_out=ysf[k])
                    nc.scalar.activation(ysb[k], ysf[k], AF.Copy)
                psum_sv = [psBv.tile([1, NB], F32, tag="vv", name=f"sv{h}")
                           for h in range(2)]
                for k in range(KD):
                    for h in range(2):
                        nc.tensor.matmul(psum_sv[h], ysb[k],
                                         wvs[k][:, h * NB:(h + 1) * NB],
                                         start=(k == 0), stop=(k == KD - 1))
                sumv_sb = small.tile([1, D], BF, tag="sumv", name="sumv")
                for h in range(2):
                    nc.vector.tensor_add(sumv_sb[:, h * NB:(h + 1) * NB],
                                         psum_sv[h],
                                         bvs_row[:, h * NB:(h + 1) * NB])
                # broadcast the row across all 128 partitions (rank-1 matmul)
                psum_svr = psBv.tile([P, D], F32, tag="vv", name="svr")
                for h in range(2):
                    nc.tensor.matmul(psum_svr[:, h * NB:(h + 1) * NB], ones_r,
                                     sumv_sb[:, h * NB:(h + 1) * NB],
                                     start=True, stop=True)
                nc.scalar.activation(sumv_rep, psum_svr, AF.Copy)

            # ---------------- Stage C: attention ----------------
            with tc.tile_pool(name="pE", bufs=1) as pE, \
                 tc.tile_pool(name="pEt", bufs=2) as pEt, \
                 tc.tile_pool(name="psCs", bufs=2, space="PSUM") as psCs, \
                 tc.tile_pool(name="psCo", bufs=2, space="PSUM") as psCo:
                em1 = alloc_pairs(pE, NP, S, "em")
                # Em1^T = exp(scale * k @ q^T) - 1, fp8 pair-packed over kv.
                # Wide (1024-col) psum tiles/drains keep the scalar engine
                # well under the PE rate during this phase.
                for tk in range(NT):
                    for qh in range(2):
                        ps = psCs.tile([P, 2 * NB], F32, tag="sc", name="sc")
                        for t in range(PD):
                            lhs = k8[t][:, :, tk * P:(tk + 1) * P]
                            for nb in range(2):
                                col = qh * 2 * NB + nb * NB
                                nc.tensor.matmul(
                                    ps[:, nb * NB:(nb + 1) * NB], lhs,
                                    q8[t][:, :, col:col + NB],
                                    start=(t == 0), stop=(t == PD - 1),
                                    perf_mode=DR)
                        et = pEt.tile([P, 2 * NB], F32, tag="et", name="et")
                        nc.scalar.activation(et, ps, AF.Exp, bias=0.0,
                                             scale=SCALE)
                        nc.vector.tensor_scalar_add(
                            em1[tk // 2][:, tk % 2,
                                         qh * 2 * NB:(qh + 1) * 2 * NB],
                            et, -1.0)
                # out rows: po = sum (E-1) v8 (DR) + sumv_rep;
                # rowsum S = 2048 + sum (E-1) via the shared-weights ones col
                # (pS tiles borrow the scores-pool psum slots, idle by now)
                for tq in range(NT):
                    po = psCo.tile([P, D], F32, tag="oo", name="oo")
                    pS = psCs.tile([P, 1], F32, tag="sc", name="ss")
                    for tp in range(NP):
                        lhs = em1[tp][:, :, tq * P:(tq + 1) * P]
                        nc.tensor.matmul(po[:, 0:NB], lhs, v8[tp][:, :, 0:NB],
                                         start=(tp == 0), stop=(tp == NP - 1),
                                         perf_mode=DR)
                        nc.tensor.matmul(po[:, NB:D], lhs, v8[tp][:, :, NB:D],
                                         start=(tp == 0), stop=(tp == NP - 1),
                                         perf_mode=DR)
                        nc.tensor.matmul(pS, lhs, ones8,
                                         start=(tp == 0), stop=(tp == NP - 1),
                                         perf_mode=DR)
                    pSt = rpool.tile([P, 1], F32, tag="pst", name="pst")
                    nc.vector.tensor_scalar_add(pSt, pS, float(S))
                    rinv = rpool.tile([P, 1], F32, tag="ri", name="ri")
                    nc.vector.reciprocal(rinv, pSt)
                    ot = outp.tile([P, D], BF, tag="ot", name="ot")
                    if tq < NT - 1:
                        tmp = tmpp.tile([P, D], F32, tag="tm", name="tm")
                        nc.vector.tensor_add(tmp, po, sumv_rep)
                        nc.scalar.activation(ot, tmp, AF.Copy, scale=rinv)
                        nc.sync.dma_start(out=out[tq * P:(tq + 1) * P, :],
                                          in_=ot)
                    else:
                        # last tile: half-width drains pipeline the final
                        # DVE -> scalar -> DMA chain to shorten the tail
                        for hh in range(2):
                            cs = slice(hh * NB, (hh + 1) * NB)
                            tmp = tmpp.tile([P, NB], F32, tag="tmh",
                                            name="tmh")
                            nc.vector.tensor_add(tmp, po[:, cs],
                                                 sumv_rep[:, cs])
                            nc.scalar.activation(ot[:, cs], tmp, AF.Copy,
                                                 scale=rinv)
                            nc.sync.dma_start(
                                out=out[tq * P:(tq + 1) * P, cs],
                                in_=ot[:, cs])

    nc.compile()
    _NC = nc
    return nc


def _pack8(w):
    """[K, N] -> DoubleRow pair-packed fp8 [K/2, 2N]:
    out[t*128+p, r*N+m] = w[(2t+r)*128+p, m]."""
    K, N = w.shape
    return np.ascontiguousarray(
        w.astype(FP8).reshape(K // 256, 2, 128, N)
        .transpose(0, 2, 1, 3).reshape(K // 2, 2 * N))


def make_in_maps(inputs):
    """Host-side prep: per-core batch shard, fp8/bf16 casts + pair packing,
    feature-major transposes of x/y, bias relayout."""
    x = np.asarray(inputs["x"])
    y = np.asarray(inputs["y"])
    shared = {}
    for k in ("W1", "W2", "W3", "W4", "Wq", "Wk"):
        shared[k] = _pack8(np.asarray(inputs[k]).astype(np.float32))
    wv = np.asarray(inputs["Wv"]).astype(np.float32)
    shared["Wv8"] = _pack8(wv)
    shared["Wv"] = np.ascontiguousarray(wv.astype(BF16))
    for k, nt in (("b1", KD), ("b2", KB), ("b3", KD), ("b4", KD),
                  ("bq", KD), ("bk", KD)):
        shared[k] = np.ascontiguousarray(
            np.asarray(inputs[k]).astype(np.float32).reshape(nt, P).T)
    bv = np.asarray(inputs["bv"]).astype(np.float32).reshape(D)
    shared["bv"] = np.ascontiguousarray(bv)
    shared["bvs"] = np.ascontiguousarray(np.float32(S) * bv)
    in_maps = []
    for b in range(x.shape[0]):
        m = dict(shared)
        xT = np.ascontiguousarray(x[b].T)
        yT = np.ascontiguousarray(y[b].T)
        m["x8"] = _pack8(xT)
        m["y8"] = _pack8(yT)
        m["yT"] = yT.astype(BF16)
        in_maps.append(m)
    return in_maps


def kernel(**inputs):
    from concourse.bass_utils import run_bass_kernel_spmd

    nc = build_nc()
    in_maps = make_in_maps(inputs)
    res = run_bass_kernel_spmd(nc, in_maps, list(range(len(in_maps))))
    return np.stack([np.asarray(r["out"]).astype(np.float32)
                     for r in res.results])


# revision 22
# speedup vs baseline: 1.0378x; 1.0378x over previous
"""Trainium2 Bass kernel for nn_CrossAttention (4-layer MLP -> cross-attention).

Sharding: data-parallel across batch B=8, one batch element per NeuronCore.

Layout strategy (per core):
  - activations flow feature-major (transposed): the MLP chain
    h_{l+1}^T = W_l^T @ h_l^T needs no transposes (W natural [K,M] = lhsT),
  - scores are computed transposed (scores^T = k @ q^T, kv on partitions) so
    Em1 = exp(scores^T)-1 feeds the attention output matmul directly as the
    stationary operand: out[q,D] = lhsT(Em1[kv,q]).T @ v8[kv,D],
  - softmax skips max-subtraction (exact by shift invariance; scores small).

Precision: ALL five GEMM groups (x-MLP/q/k, v, scores, attn@v) run in
fp8(e4m3) DoubleRow (2 contraction rows per PE pass -> 2x rate). The
attention weights E ~ 1 +- 0.1 cannot survive fp8 directly (6% steps at 1.0),
so the output is decomposed around the constant term:
    out = (sum_kv (E-1) * v8  +  sum_kv v) / (2048 + sum_kv (E-1))
The fluctuation term (E-1) ~ 0.1 and v8 are fp8 (their product errors average
over Skv=2048), while the dominant sum_kv v term is computed exactly in bf16
as (sum_kv y) @ Wv + 2048*bv via a DVE row-reduction of y^T and a small
rank-1 matmul chain. Row sums come from DR matmuls with a ones column
(sharing the attention matmuls' loaded weights); the +2048 and the 1/S
normalize happen on DVE, the final scale on the scalar engine.
All accumulation fp32 in PSUM. Output stored bf16, upcast on host.

fp8 operands are pair-packed for DoubleRow: logical feature k = (2t+r)*128+p
lives in tile t, partition p, middle index r, i.e. SBUF tiles [128, 2, N]
(weights pre-packed on host to [K/2, 2*N] so each tile is one contiguous DMA).

Drains are split between the scalar and vector engines on layers where the
PE would otherwise outrun a single drain engine (L2/L3/k: only 2-4 DR passes
per psum tile).

Everything is SBUF-resident (no DRAM round-trips); y/Wk prefetch from t=0.
"""

import sys

if "/opt/trn_rl_repo" not in sys.path:
    sys.path.insert(0, "/opt/trn_rl_repo")

import numpy as np
import ml_dtypes

P = 128
D = 1024
DB = 512
S = 2048
KD = D // P       # 8 feature tiles of 128
KB = DB // P      # 4
PD = KD // 2      # 4 fp8 pair-tiles for a 1024 contraction
PB = KB // 2      # 2 for 512
NT = S // P       # 16 token tiles
NP = NT // 2      # 8 kv pair-tiles for the attention contraction
NB = 512          # moving-operand free-dim block (psum bank width)
NBLK = S // NB    # 4 token blocks
NCORES = 8
SCALE = float(1.0 / np.sqrt(D))

BF16 = ml_dtypes.bfloat16
FP8 = ml_dtypes.float8_e4m3

_NC = None


def build_nc():
    """Build + compile the per-core Bass program (cached)."""
    global _NC
    if _NC is not None:
        return _NC

    from contextlib import ExitStack
    import concourse.bass as bass
    import concourse.tile as tile
    from concourse import bacc, mybir

    BF = mybir.dt.bfloat16
    F8 = mybir.dt.float8e4
    F32 = mybir.dt.float32
    AF = mybir.ActivationFunctionType
    ALU = mybir.AluOpType
    DR = mybir.MatmulPerfMode.DoubleRow

    nc = bacc.Bacc("TRN2", target_bir_lowering=False, debug=False,
                   num_devices=NCORES)

    def din(name, shape, dt):
        return nc.dram_tensor(name, shape, dt, kind="ExternalInput").ap()

    # fp8 operands arrive pair-packed: [K/2, 2*N]
    x8d = din("x8", [D // 2, 2 * S], F8)
    y8d = din("y8", [D // 2, 2 * S], F8)
    yTd = din("yT", [D, S], BF)
    W1d = din("W1", [D // 2, 2 * D], F8)
    W2d = din("W2", [D // 2, 2 * DB], F8)
    W3d = din("W3", [DB // 2, 2 * D], F8)
    W4d = din("W4", [D // 2, 2 * D], F8)
    Wqd = din("Wq", [D // 2, 2 * D], F8)
    Wkd = din("Wk", [D // 2, 2 * D], F8)
    Wv8d = din("Wv8", [D // 2, 2 * D], F8)
    Wvd = din("Wv", [D, D], BF)
    b1 = din("b1", [P, KD], F32)
    b2 = din("b2", [P, KB], F32)
    b3 = din("b3", [P, KD], F32)
    b4 = din("b4", [P, KD], F32)
    bq = din("bq", [P, KD], F32)
    bk = din("bk", [P, KD], F32)
    bv = din("bv", [D], F32)
    bvs = din("bvs", [D], F32)          # 2048 * bv (host-precomputed)
    out = nc.dram_tensor("out", [S, D], BF, kind="ExternalOutput").ap()

    with tile.TileContext(nc) as tc, ExitStack() as ctx:
        small = ctx.enter_context(tc.tile_pool(name="small", bufs=1))
        rpool = ctx.enter_context(tc.tile_pool(name="rpool", bufs=4))
        outp = ctx.enter_context(tc.tile_pool(name="outp", bufs=2))
        tmpp = ctx.enter_context(tc.tile_pool(name="tmpp", bufs=2))

        def load_bias(src, cols, tag):
            t = small.tile([P, cols], F32, tag=tag, name=tag)
            nc.gpsimd.dma_start(out=t, in_=src)
            return t

        b1_sb = load_bias(b1, KD, "b1")
        b2_sb = load_bias(b2, KB, "b2")
        b3_sb = load_bias(b3, KD, "b3")
        b4_sb = load_bias(b4, KD, "b4")
        bq_sb = load_bias(bq, KD, "bq")
        bk_sb = load_bias(bk, KD, "bk")

        # bv replicated across partitions for the v8 bias add
        bv_rep = small.tile([P, D], F32, tag="bvrep", name="bvrep")
        bv_bcast = bass.AP(tensor=bv.tensor, offset=bv.offset,
                           ap=[[0, P]] + list(bv.ap))
        nc.gpsimd.dma_start(out=bv_rep, in_=bv_bcast)
        # 2048*bv as a single row (for the exact sum_kv v term)
        bvs_row = small.tile([1, D], F32, tag="bvsrow", name="bvsrow")
        bvs_bcast = bass.AP(tensor=bvs.tensor, offset=bvs.offset,
                            ap=[[0, 1]] + list(bvs.ap))
        nc.gpsimd.dma_start(out=bvs_row, in_=bvs_bcast)

        # fp8 pair-packed ones column for DR row sums
        ones8 = small.tile([P, 2, 1], F8, tag="ones8", name="ones8")
        nc.vector.memset(ones8, 1.0)
        # bf16 ones row for the sumv partition broadcast
        ones_r = small.tile([1, P], BF, tag="onesr", name="onesr")
        nc.vector.memset(ones_r, 1.0)
        warm_row = small.tile([1, NB], BF, tag="warm", name="warm")
        nc.vector.memset(warm_row, 0.0)

        def alloc_pairs(pool, pairs, n, tag, dt=F8):
            """fp8 pair-packed tiles [P, 2, n]."""
            return [pool.tile([P, 2, n], dt, tag=f"{tag}{t}", name=f"{tag}{t}")
                    for t in range(pairs)]

        def load_pairs(tiles, src, n):
            for t, tl in enumerate(tiles):
                nc.sync.dma_start(
                    out=tl,
                    in_=src[t * P:(t + 1) * P, :].rearrange(
                        "p (r s) -> p r s", r=2))

        def fm_layer8(psum, src8, w8, pairs, mtiles, bias_sb, relu, dst8,
                      cw, dve_mod=0):
            """fp8 DoubleRow feature-major layer into pair-packed fp8 dst.

            cw: psum chunk width in tokens (512 or 1024). Chunks with
            (chunk_idx % 2) < dve_mod drain on the vector engine instead of
            the scalar engine (used where PE passes per chunk are few)."""
            func = AF.Relu if relu else AF.Identity
            nch = S // cw
            nsub = cw // NB
            for m in range(mtiles):
                pss = [psum.tile([P, cw], F32, tag="mm", name="mm")
                       for _ in range(nch)]
                for t in range(pairs):
                    lhs = w8[t][:, :, m * P:(m + 1) * P]
                    for c in range(nch):
                        for nb in range(nsub):
                            col = c * cw + nb * NB
                            nc.tensor.matmul(
                                pss[c][:, nb * NB:(nb + 1) * NB], lhs,
                                src8[t][:, :, col:col + NB],
                                start=(t == 0), stop=(t == pairs - 1),
                                perf_mode=DR)
                for c in range(nch):
                    dst = dst8[m // 2][:, m % 2, c * cw:(c + 1) * cw]
                    if c % 2 < dve_mod:
                        if relu:
                            nc.vector.tensor_scalar(
                                dst, pss[c], bias_sb[:, m:m + 1], 0.0,
                                op0=ALU.add, op1=ALU.max)
                        else:
                            nc.vector.tensor_scalar_add(
                                dst, pss[c], bias_sb[:, m:m + 1])
                    else:
                        nc.scalar.activation(dst, pss[c], func,
                                             bias=bias_sb[:, m:m + 1],
                                             scale=1.0)

        # ------ persistent attention operands (q8, k8, v8) + y prefetch ------
        with tc.tile_pool(name="pq", bufs=1) as pq, \
             tc.tile_pool(name="pk", bufs=1) as pk, \
             tc.tile_pool(name="pvp", bufs=1) as pvp, \
             tc.tile_pool(name="py", bufs=1) as py, \
             tc.tile_pool(name="pwk", bufs=1) as pwk, \
             tc.tile_pool(name="psv", bufs=1) as psv:
            q8 = alloc_pairs(pq, PD, S, "q8")
            k8 = alloc_pairs(pk, PD, S, "k8")
            v8 = alloc_pairs(pvp, NP, D, "v8")
            y8 = alloc_pairs(py, PD, S, "y8")
            wk8 = alloc_pairs(pwk, PD, D, "wk8")
            sumv_rep = psv.tile([P, D], F32, tag="svr", name="svr")

            # ---------------- Stage A: x-MLP -> q8 (in SBUF) ----------------
            with tc.tile_pool(name="wx", bufs=1) as wx, \
                 tc.tile_pool(name="px", bufs=1) as px, \
                 tc.tile_pool(name="phA", bufs=1) as phA, \
                 tc.tile_pool(name="phB", bufs=1) as phB, \
                 tc.tile_pool(name="psA", bufs=4, space="PSUM") as psA:
                x8 = alloc_pairs(px, PD, S, "x8")
                w18 = alloc_pairs(wx, PD, D, "w18")
                # first-needed tiles first: interleave x8 / W1 pair loads;
                # t=0 tiles split in half so the first matmul's deps land fast
                for t in range(PD):
                    xs = x8d[t * P:(t + 1) * P, :].rearrange(
                        "p (r s) -> p r s", r=2)
                    ws = W1d[t * P:(t + 1) * P, :].rearrange(
                        "p (r s) -> p r s", r=2)
                    if t == 0:
                        nc.sync.dma_start(out=x8[t][:, :, 0:NB],
                                          in_=xs[:, :, 0:NB])
                        nc.sync.dma_start(out=w18[t][:, :, 0:NB],
                                          in_=ws[:, :, 0:NB])
                        nc.sync.dma_start(out=x8[t][:, :, NB:S],
                                          in_=xs[:, :, NB:S])
                        nc.sync.dma_start(out=w18[t][:, :, NB:D],
                                          in_=ws[:, :, NB:D])
                    else:
                        nc.sync.dma_start(out=x8[t], in_=xs)
                        nc.sync.dma_start(out=w18[t], in_=ws)
                w28 = alloc_pairs(wx, PD, DB, "w28")
                load_pairs(w28, W2d, DB)
                w38 = alloc_pairs(wx, PB, D, "w38")
                load_pairs(w38, W3d, D)
                w48 = alloc_pairs(wx, PD, D, "w48")
                load_pairs(w48, W4d, D)
                wq8 = alloc_pairs(wx, PD, D, "wq8")
                load_pairs(wq8, Wqd, D)
                # y-side prefetch (queued behind stage A's needs)
                load_pairs(y8, y8d, S)
                load_pairs(wk8, Wkd, D)

                # warmup matmuls on preloaded constants: start the PE p-state
                # ramp while the first x8/W1 DMAs are still in flight
                warm_ps = psA.tile([P, 2 * NB], F32, tag="mm", name="warm_ps")
                for _ in range(9):
                    nc.tensor.matmul(warm_ps[:, 0:NB], ones_r, warm_row,
                                     start=True, stop=True)

                h18 = alloc_pairs(phA, PD, S, "ha")
                h28 = alloc_pairs(phB, PB, S, "hb")
                h38 = alloc_pairs(phA, PD, S, "ha")   # reuse phA slots
                h48 = alloc_pairs(phB, PD, S, "hb")   # grow phB to 4 pair slots
                fm_layer8(psA, x8, w18, PD, KD, b1_sb, True, h18, 2 * NB)
                fm_layer8(psA, h18, w28, PD, KB, b2_sb, True, h28, 2 * NB,
                          dve_mod=1)
                fm_layer8(psA, h28, w38, PB, KD, b3_sb, True, h38, 2 * NB,
                          dve_mod=1)
                fm_layer8(psA, h38, w48, PD, KD, b4_sb, True, h48, 2 * NB)
                fm_layer8(psA, h48, wq8, PD, KD, bq_sb, False, q8, 2 * NB)

            # ------- Stage B: y -> k8, v8 (fp8), exact sumv row (bf16) -------
            with tc.tile_pool(name="pwv", bufs=1) as pwv, \
                 tc.tile_pool(name="pys", bufs=1) as pys, \
                 tc.tile_pool(name="psBk", bufs=4, space="PSUM") as psBk, \
                 tc.tile_pool(name="psBv", bufs=2, space="PSUM") as psBv:
                # stage-B-only loads (deferred so stage A has SBUF headroom)
                wv8 = alloc_pairs(pwv, PD, D, "wv8")
                load_pairs(wv8, Wv8d, D)
                ys = [pys.tile([P, S], BF, tag=f"y{k}", name=f"y{k}")
                      for k in range(KD)]
                for k in range(KD):
                    nc.sync.dma_start(out=ys[k], in_=yTd[k * P:(k + 1) * P, :])
                wvs = [pys.tile([P, D], BF, tag=f"wv{k}", name=f"wv{k}")
                       for k in range(KD)]
                for k in range(KD):
                    nc.sync.dma_start(out=wvs[k],
                                      in_=Wvd[k * P:(k + 1) * P, :])

                # k^T in fp8 pairs; narrow psum chunks, drains split act/DVE
                fm_layer8(psBk, y8, wk8, PD, KD, bk_sb, False, k8, NB,
                          dve_mod=1)

                # v8[kv, d] = fp8(y @ Wv + bv), token-major pair-packed via DR
                for tk in range(NT):
                    pv_ = psBv.tile([P, D], F32, tag="vv", name="vv")
                    for t in range(PD):
                        lhs = y8[t][:, :, tk * P:(tk + 1) * P]
                        for nb in range(2):
                            nc.tensor.matmul(pv_[:, nb * NB:(nb + 1) * NB],
                                             lhs,
                                             wv8[t][:, :, nb * NB:(nb + 1) * NB],
                                             start=(t == 0), stop=(t == PD - 1),
                                             perf_mode=DR)
                    nc.vector.tensor_add(v8[tk // 2][:, tk % 2, :], pv_, bv_rep)

                # exact sum_kv v: ysumT = rowsum(y^T) via scalar-engine Copy
                # with accum_out (the DVE is busy with v8 drains; the scalar
                # engine idles during the v8 phase), then
                # sumv = ysumT @ Wv + 2048*bv (bf16 rank-1 chain)
                ydump = pys.tile([P, S], BF, tag="ydump", name="ydump")
                ysf = [rpool.tile([P, 1], F32, tag=f"ysf{k}", name=f"ysf{k}",
                                  bufs=1)
                       for k in range(KD)]
                ysb = [rpool.tile([P, 1], BF, tag=f"ysb{k}", name=f"ysb{k}",
                                  bufs=1)
                       for k in range(KD)]
                for k in range(KD):
                    nc.scalar.activation(ydump, ys[k], AF.Copy,
                                         accum_out=ysf[k])
                    nc.scalar.activation(ysb[k], ysf[k], AF.Copy)
                psum_sv = [psBv.tile([1, NB], F32, tag="vv", name=f"sv{h}")
                           for h in range(2)]
                for k in range(KD):
                    for h in range(2):
                        nc.tensor.matmul(psum_sv[h], ysb[k],
                                         wvs[k][:, h * NB:(h + 1) * NB],
                                         start=(k == 0), stop=(k == KD - 1))
                sumv_sb = small.tile([1, D], BF, tag="sumv", name="sumv")
                for h in range(2):
                    nc.vector.tensor_add(sumv_sb[:, h * NB:(h + 1) * NB],
                                         psum_sv[h],
                                         bvs_row[:, h * NB:(h + 1) * NB])
                # broadcast the row across all 128 partitions (rank-1 matmul)
                psum_svr = psBv.tile([P, D], F32, tag="vv", name="svr")
                for h in range(2):
                    nc.tensor.matmul(psum_svr[:, h * NB:(h + 1) * NB], ones_r,
                                     sumv_sb[:, h * NB:(h + 1) * NB],
                                     start=True, stop=True)
                nc.scalar.activation(sumv_rep, psum_svr, AF.Copy)

            # ---------------- Stage C: attention ----------------
            with tc.tile_pool(name="pE", bufs=1) as pE, \
                 tc.tile_pool(name="pEt", bufs=2) as pEt, \
                 tc.tile_pool(name="psCs", bufs=2, space="PSUM") as psCs, \
                 tc.tile_pool(name="psCo", bufs=2, space="PSUM") as psCo:
                em1 = alloc_pairs(pE, NP, S, "em")
                # Em1^T = exp(scale * k @ q^T) - 1, fp8 pair-packed over kv.
                # Wide (1024-col) psum tiles/drains keep the scalar engine
                # well under the PE rate during this phase.
                for tk in range(NT):
                    for qh in range(2):
                        ps = psCs.tile([P, 2 * NB], F32, tag="sc", name="sc")
                        for t in range(PD):
                            lhs = k8[t][:, :, tk * P:(tk + 1) * P]
                            for nb in range(2):
                                col = qh * 2 * NB + nb * NB
                                nc.tensor.matmul(
                                    ps[:, nb * NB:(nb + 1) * NB], lhs,
                                    q8[t][:, :, col:col + NB],
                                    start=(t == 0), stop=(t == PD - 1),
                                    perf_mode=DR)
                        et = pEt.tile([P, 2 * NB], F32, tag="et", name="et")
                        nc.scalar.activation(et, ps, AF.Exp, bias=0.0,
                                             scale=SCALE)
                        nc.vector.tensor_scalar_add(
                            em1[tk // 2][:, tk % 2,
                                         qh * 2 * NB:(qh + 1) * 2 * NB],
                            et, -1.0)
                # out rows: po = sum (E-1) v8 (DR) + sumv_rep;
                # rowsum S = 2048 + sum (E-1) via the shared-weights ones col
                # (pS tiles borrow the scores-pool psum slots, idle by now)
                for tq in range(NT):
                    po = psCo.tile([P, D], F32, tag="oo", name="oo")
                    pS = psCs.tile([P, 1], F32, tag="sc", name="ss")
                    for tp in range(NP):
                        lhs = em1[tp][:, :, tq * P:(tq + 1) * P]
                        nc.tensor.matmul(po[:, 0:NB], lhs, v8[tp][:, :, 0:NB],
                                         start=(tp == 0), stop=(tp == NP - 1),
                                         perf_mode=DR)
                        nc.tensor.matmul(po[:, NB:D], lhs, v8[tp][:, :, NB:D],
                                         start=(tp == 0), stop=(tp == NP - 1),
                                         perf_mode=DR)
                        nc.tensor.matmul(pS, lhs, ones8,
                                         start=(tp == 0), stop=(tp == NP - 1),
                                         perf_mode=DR)
                    pSt = rpool.tile([P, 1], F32, tag="pst", name="pst")
                    nc.vector.tensor_scalar_add(pSt, pS, float(S))
                    rinv = rpool.tile([P, 1], F32, tag="ri", name="ri")
                    nc.vector.reciprocal(rinv, pSt)
                    ot = outp.tile([P, D], BF, tag="ot", name="ot")
                    if tq < NT - 1:
                        tmp = tmpp.tile([P, D], F32, tag="tm", name="tm")
                        nc.vector.tensor_add(tmp, po, sumv_rep)
                        nc.scalar.activation(ot, tmp, AF.Copy, scale=rinv)
                        nc.sync.dma_start(out=out[tq * P:(tq + 1) * P, :],
                                          in_=ot)
                    else:
                        # last tile: half-width drains pipeline the final
                        # DVE -> scalar -> DMA chain to shorten the tail
                        for hh in range(2):
                            cs = slice(hh * NB, (hh + 1) * NB)
                            tmp = tmpp.tile([P, NB], F32, tag="tmh",
                                            name="tmh")
                            nc.vector.tensor_add(tmp, po[:, cs],
                                                 sumv_rep[:, cs])
                            nc.scalar.activation(ot[:, cs], tmp, AF.Copy,
                                                 scale=rinv)
                            nc.sync.dma_start(
                                out=out[tq * P:(tq + 1) * P, cs],
                                in_=ot[:, cs])

    nc.compile()
    _NC = nc
    return nc


def _pack8(w):
    """[K, N] -> DoubleRow pair-packed fp8 [K/2, 2N]:
    out[t*128+p, r*N+m] = w[(2t+r)*128+p, m]."""
    K, N = w.shape
    return np.ascontiguousarray(
        w.astype(FP8).reshape(K // 256, 2, 128, N)
        .transpose(0, 2, 1, 3).reshape(K // 2, 2 * N))


def make_in_maps(inputs):
    """Host-side prep: per-core batch shard, fp8/bf16 casts + pair packing,
    feature-major transposes of x/y, bias relayout."""
    x = np.asarray(inputs["x"])
    y = np.asarray(inputs["y"])
    shared = {}
    for k in ("W1", "W2", "W3", "W4", "Wq", "Wk"):
        shared[k] = _pack8(np.asarray(inputs[k]).astype(np.float32))
    wv = np.asarray(inputs["Wv"]).astype(np.float32)
    shared["Wv8"] = _pack8(wv)
    shared["Wv"] = np.ascontiguousarray(wv.astype(BF16))
    for k, nt in (("b1", KD), ("b2", KB), ("b3", KD), ("b4", KD),
                  ("bq", KD), ("bk", KD)):
        shared[k] = np.ascontiguousarray(
            np.asarray(inputs[k]).astype(np.float32).reshape(nt, P).T)
    bv = np.asarray(inputs["bv"]).astype(np.float32).reshape(D)
    shared["bv"] = np.ascontiguousarray(bv)
    shared["bvs"] = np.ascontiguousarray(np.float32(S) * bv)
    in_maps = []
    for b in range(x.shape[0]):
        m = dict(shared)
        xT = np.ascontiguousarray(x[b].T)
        yT = np.ascontiguousarray(y[b].T)
        m["x8"] = _pack8(xT)
        m["y8"] = _pack8(yT)
        m["yT"] = yT.astype(BF16)
        in_maps.append(m)
    return in_maps


def kernel(**inputs):
    from concourse.bass_utils import run_bass_kernel_spmd

    nc = build_nc()
    in_maps = make_in_maps(inputs)
    res = run_bass_kernel_spmd(nc, in_maps, list(range(len(in_maps))))
    return np.stack([np.asarray(r["out"]).astype(np.float32)
                     for r in res.results])
